# revision 37
# baseline (speedup 1.0000x reference)
"""Causal self-attention (B=4, S=2048, E=1024, H=16, hd=64) on 8 TRN2 NeuronCores.

Sharding: tensor-parallel over (batch, head-half). Core c handles batch c%4 and
heads [8*(c//4), 8*(c//4)+8) -- a 512-wide slice of the Wq/Wk/Wv columns and of
the Wo rows. Each core computes a partial [S, E] c_proj output (bf16); the host
sums the two half partials per batch and adds bo.

v3 design (HW-measured ~253-256us, from 327us v2 baseline):

Host-side input prep (same bf16 rounding the kernel used to do on-chip):
  x pre-transposed to chunk-major xT layout [sc, e-in-tile, kt, s'] -- no
  on-chip transposes or PSUM-bounce copies at all; weights bf16 and
  pre-swizzled to the SBUF layout [p, ko, m] (contiguous 8KB-per-partition
  DMA lines); biases packed into one [128, 8+512] tensor (bq tiles
  pre-scaled by 1/8, bk tiles, bv broadcast) -- a single fast DMA instead of
  three 512-descriptor gathers that used to block the queue head ~10us.

Prologue: ~32 dummy matmuls (on a DVE-memset tile, independent of the slow
  gpsimd queue) warm the HAM clock gate (cold PE = 1.2 GHz) while x chunk 0
  and wq stream kt-halved on the two parallel hwdge DMA queues (x/out on
  sync, weights on Act), so the first q accumulation starts ~12us.

Attention steps (query chunk ic, head pair hp, key tile jt): scores
  S_T = kT-stationary @ qT into PSUM; both heads' K=64 matmuls are
  row-tiled (auto tile_position from base partitions 0/64) and run
  CONCURRENTLY in the PE array. Scores are emitted in PAIRS of steps, then
  the two staged flushes of the previous pair run: pairing gives every
  LDWEIGHTS a drained weight-buffer slot to prefetch into (weight-changing
  matmuls otherwise serialize LDW behind the in-flight matmul drain,
  ~230ns/step). Flush: one wide Act exp -> bf16 P tile; DVE trimask muls on
  diagonal tiles; PV accumulates into psy [128, 2, 512].

Softmax normalization without any PE broadcast: v2's ones-column is widened
  to 64 ones-columns [v(64) | ones(64)], so the PV matmul itself replicates
  the softmax sums into psy rows 64:127 (matmul cost depends only on the
  moving free dim -- the replication is free). Norm pipeline, one stage per
  pair: DVE copy shifts sums rows 64:128 down to base partition 0 (plain
  DVE copies may shift partitions; the custom reciprocal op and two-source
  ops must be base-aligned), DVE reciprocal_approx_fast, DVE muls into yT.

Work items (next-chunk q/k/v projections, c_proj groups) are paced evenly
  across the attention pairs of each stretch: ic0->prep sc1, ic1->prep sc2,
  ic2->prep sc3 + cproj st0-3, ic3->cproj st4-11 (ic2/ic3 are Act(exp)-
  bound, so PE work there is free). Tail: cproj st12-15 pre-accumulates its
  ptd<3 contributions (st15 in the freed psy bank) overlapped with the last
  head-pair's norm chain; final output DMAs split across both hwdge queues.

PSUM: shared [128,2,512] pool (3 bufs, 6 banks) serves scores, projection
  accumulators and c_proj; psy 1 buf (2 banks) = 8 banks exactly. All matmul
  operands bf16 (full PE rate), fp32 accumulation, bf16 partial outputs
  (host upcasts and sums).
"""

import numpy as np

import concourse.bass as bass
from concourse import bacc
import concourse.mybir as mybir
import concourse.tile as tile
from concourse.bass_utils import run_bass_kernel_spmd
from concourse.masks import make_identity

# Problem dims (hardcoded per contract)
B, S, E, H, HD = 4, 2048, 1024, 16, 64
NCORES = 8
EH = 512            # per-core slice of E (8 heads)
NHP = 4             # head pairs per core (2 heads share a 128-partition tile)
NPT = EH // 128     # 4 partition tiles of the per-core head slice
NKT = E // 128      # 8 contraction tiles over E
NST = S // 128      # 16 s-tiles
NIC = S // 512      # 4 query chunks
SCALE = 1.0 / np.sqrt(HD)

F32 = mybir.dt.float32
F32R = mybir.dt.float32r
BF16 = mybir.dt.bfloat16

_CACHED_NC = {}


def build_bass(mode="mixed"):
    """Build the single-core SPMD Bass program (same program on all 8 cores)."""
    nc = bacc.Bacc()
    # x arrives host-pre-transposed, chunk-major: [sc, e-in-tile, kt, s']
    x_h = nc.declare_dram_parameter("x", [NIC, 128, NKT, 512], BF16, isOutput=False)
    # weights pre-swizzled on host to the SBUF layout [p, ko, m] so the
    # load is one contiguous-per-partition DMA (8KB lines, no descriptor
    # blowup from strided gathers)
    wq_h = nc.declare_dram_parameter("wq", [128, NKT, EH], BF16, isOutput=False)
    wk_h = nc.declare_dram_parameter("wk", [128, NKT, EH], BF16, isOutput=False)
    wv_h = nc.declare_dram_parameter("wv", [128, NKT, EH], BF16, isOutput=False)
    wo_h = nc.declare_dram_parameter("wo", [128, NPT, E], BF16, isOutput=False)
    # host-packed biases, one contiguous fast DMA: cols 0:4 bq(tiles,
    # pre-scaled), 4:8 bk, 8:520 bv broadcast across partitions
    bz_h = nc.declare_dram_parameter("bz", [128, 8 + EH], F32, isOutput=False)
    out_h = nc.declare_dram_parameter("out", [S, E], BF16, isOutput=True)

    with tile.TileContext(nc) as tc:
        _build_body(nc, tc, x_h, wq_h, wk_h, wv_h, wo_h, bz_h, out_h)
    if not nc.is_finalized():
        nc.finalize()
    return nc


def _build_body(nc, tc, x_h, wq_h, wk_h, wv_h, wo_h, bz_h, out_h):
    import contextlib

    Exp = mybir.ActivationFunctionType.Exp
    Ident = mybir.ActivationFunctionType.Identity
    Mult = mybir.AluOpType.mult
    Add = mybir.AluOpType.add

    with contextlib.ExitStack() as ctx:
        const = ctx.enter_context(tc.tile_pool(name="const", bufs=1))
        big = ctx.enter_context(tc.tile_pool(name="big", bufs=1))
        ps = ctx.enter_context(tc.tile_pool(name="ps", bufs=3, space="PSUM"))
        ps_y = ctx.enter_context(tc.tile_pool(name="ps_y", bufs=1, space="PSUM"))
        xs_pool = ctx.enter_context(tc.tile_pool(name="xs", bufs=4))
        pt_pool = ctx.enter_context(tc.tile_pool(name="ptp", bufs=4))
        yu_pool = ctx.enter_context(tc.tile_pool(name="yup", bufs=2))
        rb_pool = ctx.enter_context(tc.tile_pool(name="rbp", bufs=2))
        sl_pool = ctx.enter_context(tc.tile_pool(name="slp", bufs=1))
        out_pool = ctx.enter_context(tc.tile_pool(name="outp", bufs=2))

        # upper-triangular (keep q_idx >= k_idx) multiplicative mask for the
        # 128-wide diagonal strip of each causal block
        trimask = const.tile([128, 128], BF16, name="trimask")
        nc.gpsimd.memset(trimask[:], 1.0)
        nc.gpsimd.affine_select(
            out=trimask[:],
            in_=trimask[:],
            compare_op=mybir.AluOpType.is_ge,
            fill=0.0,
            base=0,
            pattern=[[1, 128]],
            channel_multiplier=-1,
        )
        bz_sb = const.tile([128, 8 + EH], F32, name="bz_sb")
        bq_sb = bz_sb[:, 0:NPT]
        bk_sb = bz_sb[:, NPT : 2 * NPT]
        bvb = bz_sb[:, 8 : 8 + EH]

        # Resident bf16 weights (BIR verifier requires matching matmul operand
        # dtypes when f32/f32r is involved, so the compute path is all-bf16).
        wq_sb = big.tile([128, NKT, EH], BF16, name="wq_sb")
        wk_sb = big.tile([128, NKT, EH], BF16, name="wk_sb")
        wv_sb = big.tile([128, NKT, EH], BF16, name="wv_sb")
        wo_sb = big.tile([128, NPT, E], BF16, name="wo_sb")

        def load_weight(w_h, w_sb, eng=None):
            # weights arrive bf16 + pre-swizzled from the host: straight DMA
            (eng or nc.scalar).dma_start(w_sb[:], w_h[:])

        # Long-lived activation tensors
        # xT chunk-major [e-in-tile, sc, kt, s']: DMA'd straight from the
        # host-pre-transposed x (no on-chip transposes at all)
        xT = big.tile([128, NIC, NKT, 512], BF16, name="xT")
        qT = big.tile([128, NHP, S], BF16, name="qT")   # [d(2 heads), hp, s]
        kT = big.tile([128, NHP, S], BF16, name="kT")
        # v2: [s-in-tile, st, hp, head-in-pair, 2*hd]; cols 64:128 all ones, so
        # the PV matmul replicates the softmax sums across psum rows 64:127
        # (matmul cost depends only on the moving free dim, so this is free)
        # and no partition-broadcast is needed for the normalization.
        v2 = big.tile([128, NST, NHP, 2, 2 * HD], BF16, name="v2")
        nc.gpsimd.memset(v2[:, :, :, :, HD : 2 * HD], 1.0)
        yT = big.tile([128, NPT, S], BF16, name="yT")

        # ---------------- emission helpers ----------------
        def issue_x_dma(sc):
            nc.sync.dma_start(xT[:, sc, :, :], x_h[:][sc])

        cp_rot = [0]

        def emit_qk(which, pt, sc, eng="alt"):
            w_sb, b_sb = (wq_sb, bq_sb) if which == "q" else (wk_sb, bk_sb)
            outT = qT if which == "q" else kT
            acc2 = ps.tile([128, 2, 512], F32, tag="grp", name="acc2")
            acc = acc2[:, 0, :]
            for kt in range(NKT):
                nc.tensor.matmul(
                    acc,
                    lhsT=w_sb[:, kt, pt * 128 : (pt + 1) * 128],
                    rhs=xT[:, sc, kt, :],
                    start=(kt == 0),
                    stop=(kt == NKT - 1),
                )
            dst = outT[:, pt, sc * 512 : (sc + 1) * 512]
            if eng == "alt":
                eng = "act" if cp_rot[0] == 0 else "dve"
                cp_rot[0] = 1 - cp_rot[0]
            if eng == "act":
                nc.scalar.activation(
                    dst, acc, Ident,
                    bias=b_sb[:, pt : pt + 1],
                    scale=float(SCALE) if which == "q" else 1.0,
                )
            elif which == "q":
                # q = acc*scale + bias (bias pre-scaled on host)
                nc.vector.tensor_scalar(
                    dst, acc, float(SCALE), b_sb[:, pt : pt + 1], Mult, Add
                )
            else:
                nc.vector.tensor_scalar_add(dst, acc, b_sb[:, pt : pt + 1])

        def emit_v(st):
            acc2 = ps.tile([128, 2, 512], F32, tag="grp", name="vacc")
            acc = acc2[:, 0, :]
            for kt in range(NKT):
                nc.tensor.matmul(
                    acc,
                    lhsT=xT[:, st // 4, kt, (st % 4) * 128 : (st % 4) * 128 + 128],
                    rhs=wv_sb[:, kt, :],
                    start=(kt == 0),
                    stop=(kt == NKT - 1),
                )
            nc.vector.tensor_add(
                v2[:, st, :, :, 0:HD],
                acc.rearrange("p (a b c) -> p a b c", a=NHP, b=2),
                bvb.rearrange("p (a b c) -> p a b c", a=NHP, b=2),
            )

        def emit_cproj_st(st, direct=False):
            cacc = ps.tile([128, 2, 512], F32, tag="grp", name="cacc")
            for ec in range(2):
                for ptd in range(NPT):
                    nc.tensor.matmul(
                        cacc[:, ec, :],
                        lhsT=yT[:, ptd, st * 128 : (st + 1) * 128],
                        rhs=wo_sb[:, ptd, ec * 512 : (ec + 1) * 512],
                        start=(ptd == 0),
                        stop=(ptd == NPT - 1),
                    )
            if direct:
                # tail path: 4-deep staging from the (now idle) xs pool kills
                # the copy->DMA WAR serialization; alternate Act/DVE copies
                ot = xs_pool.tile([128, E], BF16, tag="xs")
                otv = ot[:].rearrange("p (a b) -> p a b", a=2)
                if st % 2 == 0:
                    nc.scalar.activation(otv, cacc[:], Ident)
                else:
                    nc.vector.tensor_copy(otv, cacc[:])
                nc.scalar.dma_start(out_h[:][st * 128 : (st + 1) * 128, :], ot[:])
            else:
                for ec in range(2):
                    ot = out_pool.tile([128, 512], BF16, tag="ot")
                    nc.vector.tensor_copy(ot[:], cacc[:, ec, :])
                    nc.sync.dma_start(
                        out_h[:][st * 128 : (st + 1) * 128, ec * 512 : (ec + 1) * 512],
                        ot[:],
                    )

        # ---- attention pipeline (global cross-iteration staging) ----
        stage = []          # staged (flush_fn) entries, depth 2
        pending_norm = []   # (yu, hp, ic) awaiting sums partition-shift DMA
        norm_sh = []        # (yu, slo, hp, ic) shift done, recip pending
        norm_rc = []        # (yu, rrb, hp, ic) recip done, muls pending

        def make_flush(jt, ps_s, pt_t, psy, ic, hp, first, last):
            def flush():
                r = jt - 4 * ic
                w0 = 128 * r if r >= 0 else 0
                W = 512 - w0
                nc.scalar.activation(pt_t[:, :, w0:512], ps_s[:, :, 0:W], Exp)
                if r >= 0:
                    for hh in range(2):
                        nc.vector.tensor_mul(
                            pt_t[:, hh, w0 : w0 + 128],
                            pt_t[:, hh, w0 : w0 + 128],
                            trimask[:],
                        )
                for hh in range(2):
                    nc.tensor.matmul(
                        psy[:, hh, w0:512],
                        lhsT=v2[:, jt, hp, hh, :],
                        rhs=pt_t[:, hh, w0:512],
                        start=first,
                        stop=last,
                    )
                if last:
                    # free the PSUM accumulator fast: per-hh copies running in
                    # parallel on Act and DVE; rows 0:64 = y, rows 64:128 =
                    # softmax sums replicated by the v2 ones block
                    yu = yu_pool.tile([128, 2, 512], F32, tag="yu", name="yu")
                    if ic >= 2:
                        nc.vector.tensor_copy(yu[:, 0, :], psy[:, 0, :])
                        nc.vector.tensor_copy(yu[:, 1, :], psy[:, 1, :])
                    else:
                        nc.scalar.activation(yu[:, 0, :], psy[:, 0, :], Ident)
                        nc.vector.tensor_copy(yu[:, 1, :], psy[:, 1, :])
                    pending_norm.append((yu, hp, ic))
            return flush

        def norm_tick():
            # advance the oldest norm-pipeline stage by one step.  DVE ops
            # need both sources on the same base partition (custom recip must
            # be fully unshifted), so a SBUF->SBUF DMA moves the replicated
            # sums rows 64:128 down to base 0 first.
            if norm_rc:
                yu, rrb, hp_, ic_ = norm_rc.pop(0)
                for hh in range(2):
                    nc.vector.tensor_mul(
                        yT[hh * 64 : hh * 64 + 64, hp_, ic_ * 512 : (ic_ + 1) * 512],
                        yu[0:HD, hh, :],
                        rrb[:, hh, :],
                    )
            elif norm_sh:
                yu, slo, hp_, ic_ = norm_sh.pop(0)
                rrb = rb_pool.tile([HD, 2, 512], F32, tag="rrb", name="rrb")
                nc.vector.reciprocal_approx_fast(rrb[:, :, :], slo[:, :, :])
                norm_rc.append((yu, rrb, hp_, ic_))
            elif pending_norm:
                yu, hp_, ic_ = pending_norm.pop(0)
                slo = sl_pool.tile([HD, 2, 512], F32, tag="slo", name="slo")
                nc.vector.tensor_copy(slo[:, :, :], yu[HD : 2 * HD, :, :])
                norm_sh.append((yu, slo, hp_, ic_))

        # ---------------- prologue: sc=0 inputs + weights + proj ----------------
        # DMA issue order is consumption order: the sync queue is FIFO, so x
        # tiles whose staging buffer WARs on a not-yet-run transpose would
        # block later weight loads if issued too early
        _sc = nc.enter_named_scope("fused", False)[0]
        # HAM warm-up: keep the PE continuously busy >3.4us before the first
        # real work so the clock gate opens to 2.4 GHz by the time x arrives.
        # The warm tile is DVE-memset so the warm-ups do NOT wait behind the
        # slow gpsimd queue (trimask select sits after the 7us v2-ones
        # memset there).
        wtile = const.tile([128, 128], BF16, name="wtile")
        nc.vector.memset(wtile[:], 1.0)
        warm = ps.tile([128, 2, 512], F32, tag="grp", name="warm")
        for i in range(32):
            nc.tensor.matmul(
                warm[:, 0, 0:128], lhsT=wtile[:], rhs=wtile[:],
                start=True, stop=True,
            )
        # x chunks on the sync queue (chunk 0 gates the prologue chain);
        # weights stream in parallel on the Act hwdge queue
        # kt-halved first loads: q's kt0-3 matmuls start as soon as the
        # first halves of x chunk 0 and wq land
        nc.sync.dma_start(xT[:, 0, 0:4, :], x_h[:][0, :, 0:4, :])
        nc.scalar.dma_start(wq_sb[:, 0:4, :], wq_h[:][:, 0:4, :])
        nc.sync.dma_start(xT[:, 0, 4:8, :], x_h[:][0, :, 4:8, :])
        nc.scalar.dma_start(wq_sb[:, 4:8, :], wq_h[:][:, 4:8, :])
        nc.scalar.dma_start(wk_sb[:, 0:4, :], wk_h[:][:, 0:4, :])
        issue_x_dma(1)
        nc.sync.dma_start(bz_sb[:], bz_h[:])
        nc.scalar.dma_start(wk_sb[:, 4:8, :], wk_h[:][:, 4:8, :])
        load_weight(wv_h, wv_sb)
        issue_x_dma(2)
        issue_x_dma(3)
        load_weight(wo_h, wo_sb)
        for pt in range(NPT):
            emit_qk("q", pt, 0)
        for pt in range(NPT):
            emit_qk("k", pt, 0)
        for st in range(4):
            emit_v(st)

        # ---------------- fused attention + next-chunk projection ----------------
        for ic in range(NIC):
            items = []
            if ic < 3:
                scn = ic + 1
                eng = "alt" if ic < 2 else "dve"
                for pt in range(NPT):
                    items.append(
                        lambda pt=pt, scn=scn, eng=eng: emit_qk("q", pt, scn, eng)
                    )
                for pt in range(NPT):
                    items.append(
                        lambda pt=pt, scn=scn, eng=eng: emit_qk("k", pt, scn, eng)
                    )
                for st in range(4 * scn, 4 * scn + 4):
                    items.append(lambda st=st: emit_v(st))
            if ic == 2:
                for st in range(0, 4):
                    items.append(lambda st=st: emit_cproj_st(st))
            if ic == 3:
                for st in range(4, 12):
                    items.append(lambda st=st: emit_cproj_st(st))

            # Scores are emitted in PAIRS of steps, then the two staged
            # flushes for the previous pair run.  Pairing gives every
            # LDWEIGHTS a drained weight-buffer slot to prefetch into
            # (consecutive full-width/row-tiled stationaries otherwise
            # serialize LDW behind the in-flight matmul drain).
            # Item emission between pairs only (not at pair boundaries near
            # the psy drain).
            njt = 4 * ic + 4
            npair = njt // 2
            n_allowed = NHP * npair
            adone = 0
            emitted = 0
            for hp in range(NHP):
                psy = ps_y.tile([128, 2, 512], F32, tag="y", name="psy")
                for pj in range(npair):
                    for jt in (2 * pj, 2 * pj + 1):
                        pos = jt
                        r = jt - 4 * ic
                        w0 = 128 * r if r >= 0 else 0
                        W = 512 - w0
                        ps_s = ps.tile([128, 2, 512], F32, tag="grp")
                        for hh in range(2):
                            base = hh * 64
                            nc.tensor.matmul(
                                ps_s[:, hh, 0:W],
                                lhsT=kT[base : base + 64, hp, jt * 128 : (jt + 1) * 128],
                                rhs=qT[base : base + 64, hp, ic * 512 + w0 : (ic + 1) * 512],
                                start=True,
                                stop=True,
                            )
                        pt_t = pt_pool.tile([128, 2, 512], BF16, tag="pt")
                        stage.append(
                            make_flush(jt, ps_s, pt_t, psy, ic, hp,
                                       pos == 0, pos == njt - 1)
                        )
                    while len(stage) > 2:
                        stage.pop(0)()
                    norm_tick()
                    if True:
                        adone += 1
                        target = (adone * len(items) + n_allowed - 1) // n_allowed
                        while items and emitted < target:
                            items[emitted]()
                            emitted += 1
            while items and emitted < len(items):
                items[emitted]()
                emitted += 1

        # drain: overlap the last head-pair's norm chain with the tail
        # cproj by pre-accumulating the ptd<3 contributions (hp0-2 normed).
        # st15 borrows the freed psy bank; final DMAs split across queues.
        while stage:
            stage.pop(0)()
        norm_tick()
        norm_tick()
        cacc_t = {}
        for st in (12, 13, 14, 15):
            pool_ = ps if st < 15 else ps_y
            cacc = pool_.tile([128, 2, 512], F32, tag="grp" if st < 15 else "y",
                              name="cacc")
            cacc_t[st] = cacc
            for ec in range(2):
                for ptd in range(3):
                    nc.tensor.matmul(
                        cacc[:, ec, :],
                        lhsT=yT[:, ptd, st * 128 : (st + 1) * 128],
                        rhs=wo_sb[:, ptd, ec * 512 : (ec + 1) * 512],
                        start=(ptd == 0),
                        stop=False,
                    )
            norm_tick()
        while pending_norm or norm_sh or norm_rc:
            norm_tick()
        for st in (12, 13, 14, 15):
            cacc = cacc_t[st]
            for ec in range(2):
                nc.tensor.matmul(
                    cacc[:, ec, :],
                    lhsT=yT[:, 3, st * 128 : (st + 1) * 128],
                    rhs=wo_sb[:, 3, ec * 512 : (ec + 1) * 512],
                    start=False,
                    stop=True,
                )
            ot = xs_pool.tile([128, E], BF16, tag="xs")
            otv = ot[:].rearrange("p (a b) -> p a b", a=2)
            if st % 2 == 0:
                nc.scalar.activation(otv, cacc[:], Ident)
            else:
                nc.vector.tensor_copy(otv, cacc[:])
            deng = nc.sync if st % 2 == 0 else nc.scalar
            deng.dma_start(out_h[:][st * 128 : (st + 1) * 128, :], ot[:])
        nc.leave_named_scope("fused", _sc, False)

def _get_nc(mode="mixed"):
    if mode not in _CACHED_NC:
        _CACHED_NC[mode] = build_bass(mode)
    return _CACHED_NC[mode]


def make_in_maps(x, Wq, bq, Wk, bk, Wv, bv, Wo, bo):
    import ml_dtypes

    bf16 = ml_dtypes.bfloat16
    in_maps = []
    for c in range(NCORES):
        b = c % B
        half = c // B
        sl = slice(half * EH, (half + 1) * EH)
        in_maps.append(
            {
                "x": np.ascontiguousarray(
                    x[b]
                    .reshape(NIC, 512, NKT, 128)
                    .transpose(0, 3, 2, 1)
                ).astype(bf16),
                "wq": np.ascontiguousarray(
                    Wq[:, sl].reshape(NKT, 128, EH).transpose(1, 0, 2)
                ).astype(bf16),
                "wk": np.ascontiguousarray(
                    Wk[:, sl].reshape(NKT, 128, EH).transpose(1, 0, 2)
                ).astype(bf16),
                "wv": np.ascontiguousarray(
                    Wv[:, sl].reshape(NKT, 128, EH).transpose(1, 0, 2)
                ).astype(bf16),
                "wo": np.ascontiguousarray(
                    Wo[sl, :].reshape(NPT, 128, E).transpose(1, 0, 2)
                ).astype(bf16),
                "bz": np.concatenate(
                    [
                        (bq[sl] * np.float32(SCALE)).reshape(NPT, 128).T,
                        bk[sl].reshape(NPT, 128).T,
                        np.tile(bv[sl], (128, 1)),
                    ],
                    axis=1,
                ).astype(np.float32),
            }
        )
    return in_maps


def assemble(results, bo):
    out = np.empty((B, S, E), dtype=np.float32)
    for b in range(B):
        out[b] = (
            results[b]["out"].astype(np.float32)
            + results[b + B]["out"].astype(np.float32)
            + bo[None, :]
        )
    return out


def kernel(x, Wq, bq, Wk, bk, Wv, bv, Wo, bo, _trace=False, _mode="mixed"):
    x = np.asarray(x, dtype=np.float32)
    Wq = np.asarray(Wq, dtype=np.float32)
    bq = np.asarray(bq, dtype=np.float32)
    Wk = np.asarray(Wk, dtype=np.float32)
    bk = np.asarray(bk, dtype=np.float32)
    Wv = np.asarray(Wv, dtype=np.float32)
    bv = np.asarray(bv, dtype=np.float32)
    Wo = np.asarray(Wo, dtype=np.float32)
    bo = np.asarray(bo, dtype=np.float32)

    nc = _get_nc(_mode)
    in_maps = make_in_maps(x, Wq, bq, Wk, bk, Wv, bv, Wo, bo)
    res = run_bass_kernel_spmd(nc, in_maps, list(range(NCORES)), trace=_trace)
    out = assemble(res.results, bo)
    if _trace:
        return out, res
    return out



# revision 39
# speedup vs baseline: 1.0142x; 1.0142x over previous
"""Causal self-attention (B=4, S=2048, E=1024, H=16, hd=64) on 8 TRN2 NeuronCores.

Sharding: tensor-parallel over (batch, head-half). Core c handles batch c%4 and
heads [8*(c//4), 8*(c//4)+8) -- a 512-wide slice of the Wq/Wk/Wv columns and of
the Wo rows. Each core computes a partial [S, E] c_proj output (bf16); the host
sums the two half partials per batch and adds bo.

v3 design (HW-measured ~253-256us, from 327us v2 baseline):

Host-side input prep (same bf16 rounding the kernel used to do on-chip):
  x pre-transposed to chunk-major xT layout [sc, e-in-tile, kt, s'] -- no
  on-chip transposes or PSUM-bounce copies at all; weights bf16 and
  pre-swizzled to the SBUF layout [p, ko, m] (contiguous 8KB-per-partition
  DMA lines); biases packed into one [128, 8+512] tensor (bq tiles
  pre-scaled by 1/8, bk tiles, bv broadcast) -- a single fast DMA instead of
  three 512-descriptor gathers that used to block the queue head ~10us.

Prologue: ~32 dummy matmuls (on a DVE-memset tile, independent of the slow
  gpsimd queue) warm the HAM clock gate (cold PE = 1.2 GHz) while x chunk 0
  and wq stream kt-halved on the two parallel hwdge DMA queues (x/out on
  sync, weights on Act), so the first q accumulation starts ~12us.

Attention steps (query chunk ic, head pair hp, key tile jt): scores
  S_T = kT-stationary @ qT into PSUM; both heads' K=64 matmuls are
  row-tiled (auto tile_position from base partitions 0/64) and run
  CONCURRENTLY in the PE array. Scores are emitted in PAIRS of steps, then
  the two staged flushes of the previous pair run: pairing gives every
  LDWEIGHTS a drained weight-buffer slot to prefetch into (weight-changing
  matmuls otherwise serialize LDW behind the in-flight matmul drain,
  ~230ns/step). Flush: one wide Act exp -> bf16 P tile; DVE trimask muls on
  diagonal tiles; PV accumulates into psy [128, 2, 512].

Softmax normalization without any PE broadcast: v2's ones-column is widened
  to 64 ones-columns [v(64) | ones(64)], so the PV matmul itself replicates
  the softmax sums into psy rows 64:127 (matmul cost depends only on the
  moving free dim -- the replication is free). Norm pipeline, one stage per
  pair: DVE copy shifts sums rows 64:128 down to base partition 0 (plain
  DVE copies may shift partitions; the custom reciprocal op and two-source
  ops must be base-aligned), DVE reciprocal_approx_fast, DVE muls into yT.

Work items (next-chunk q/k/v projections, c_proj groups) are paced evenly
  across the attention pairs of each stretch: ic0->prep sc1, ic1->prep sc2,
  ic2->prep sc3 + cproj st0-3, ic3->cproj st4-11 (ic2/ic3 are Act(exp)-
  bound, so PE work there is free). Tail: cproj st12-15 pre-accumulates its
  ptd<3 contributions (st15 in the freed psy bank) overlapped with the last
  head-pair's norm chain; final output DMAs split across both hwdge queues.

PSUM: shared [128,2,512] pool (3 bufs, 6 banks) serves scores, projection
  accumulators and c_proj; psy 1 buf (2 banks) = 8 banks exactly. All matmul
  operands bf16 (full PE rate), fp32 accumulation, bf16 partial outputs
  (host upcasts and sums).
"""

import numpy as np

import concourse.bass as bass
from concourse import bacc
import concourse.mybir as mybir
import concourse.tile as tile
from concourse.bass_utils import run_bass_kernel_spmd
from concourse.masks import make_identity

# Problem dims (hardcoded per contract)
B, S, E, H, HD = 4, 2048, 1024, 16, 64
NCORES = 8
EH = 512            # per-core slice of E (8 heads)
NHP = 4             # head pairs per core (2 heads share a 128-partition tile)
NPT = EH // 128     # 4 partition tiles of the per-core head slice
NKT = E // 128      # 8 contraction tiles over E
NST = S // 128      # 16 s-tiles
NIC = S // 512      # 4 query chunks
SCALE = 1.0 / np.sqrt(HD)

F32 = mybir.dt.float32
F32R = mybir.dt.float32r
BF16 = mybir.dt.bfloat16

_CACHED_NC = {}


def build_bass(mode="mixed"):
    """Build the single-core SPMD Bass program (same program on all 8 cores)."""
    nc = bacc.Bacc()
    # x arrives host-pre-transposed, chunk-major: [sc, e-in-tile, kt, s']
    x_h = nc.declare_dram_parameter("x", [NIC, 128, NKT, 512], BF16, isOutput=False)
    # weights pre-swizzled on host to the SBUF layout [p, ko, m] so the
    # load is one contiguous-per-partition DMA (8KB lines, no descriptor
    # blowup from strided gathers)
    wq_h = nc.declare_dram_parameter("wq", [128, NKT, EH], BF16, isOutput=False)
    wk_h = nc.declare_dram_parameter("wk", [128, NKT, EH], BF16, isOutput=False)
    wv_h = nc.declare_dram_parameter("wv", [128, NKT, EH], BF16, isOutput=False)
    wo_h = nc.declare_dram_parameter("wo", [128, NPT, E], BF16, isOutput=False)
    # host-packed biases, one contiguous fast DMA: cols 0:4 bq(tiles,
    # pre-scaled), 4:8 bk, 8:520 bv broadcast across partitions
    bz_h = nc.declare_dram_parameter("bz", [128, 8 + EH], F32, isOutput=False)
    out_h = nc.declare_dram_parameter("out", [S, E], BF16, isOutput=True)

    with tile.TileContext(nc) as tc:
        _build_body(nc, tc, x_h, wq_h, wk_h, wv_h, wo_h, bz_h, out_h)
    if not nc.is_finalized():
        nc.finalize()
    return nc


def _build_body(nc, tc, x_h, wq_h, wk_h, wv_h, wo_h, bz_h, out_h):
    import contextlib

    Exp = mybir.ActivationFunctionType.Exp
    Ident = mybir.ActivationFunctionType.Identity
    Mult = mybir.AluOpType.mult
    Add = mybir.AluOpType.add

    with contextlib.ExitStack() as ctx:
        const = ctx.enter_context(tc.tile_pool(name="const", bufs=1))
        big = ctx.enter_context(tc.tile_pool(name="big", bufs=1))
        ps = ctx.enter_context(tc.tile_pool(name="ps", bufs=3, space="PSUM"))
        ps_y = ctx.enter_context(tc.tile_pool(name="ps_y", bufs=1, space="PSUM"))
        xs_pool = ctx.enter_context(tc.tile_pool(name="xs", bufs=4))
        pt_pool = ctx.enter_context(tc.tile_pool(name="ptp", bufs=4))
        yu_pool = ctx.enter_context(tc.tile_pool(name="yup", bufs=2))
        rb_pool = ctx.enter_context(tc.tile_pool(name="rbp", bufs=2))
        sl_pool = ctx.enter_context(tc.tile_pool(name="slp", bufs=1))
        out_pool = ctx.enter_context(tc.tile_pool(name="outp", bufs=2))

        # upper-triangular (keep q_idx >= k_idx) multiplicative mask for the
        # 128-wide diagonal strip of each causal block
        trimask = const.tile([128, 128], BF16, name="trimask")
        nc.gpsimd.memset(trimask[:], 1.0)
        nc.gpsimd.affine_select(
            out=trimask[:],
            in_=trimask[:],
            compare_op=mybir.AluOpType.is_ge,
            fill=0.0,
            base=0,
            pattern=[[1, 128]],
            channel_multiplier=-1,
        )
        bz_sb = const.tile([128, 8 + EH], F32, name="bz_sb")
        bq_sb = bz_sb[:, 0:NPT]
        bk_sb = bz_sb[:, NPT : 2 * NPT]
        bvb = bz_sb[:, 8 : 8 + EH]

        # Resident bf16 weights (BIR verifier requires matching matmul operand
        # dtypes when f32/f32r is involved, so the compute path is all-bf16).
        wq_sb = big.tile([128, NKT, EH], BF16, name="wq_sb")
        wk_sb = big.tile([128, NKT, EH], BF16, name="wk_sb")
        wv_sb = big.tile([128, NKT, EH], BF16, name="wv_sb")
        wo_sb = big.tile([128, NPT, E], BF16, name="wo_sb")

        def load_weight(w_h, w_sb, eng=None):
            # weights arrive bf16 + pre-swizzled from the host: straight DMA
            (eng or nc.scalar).dma_start(w_sb[:], w_h[:])

        # Long-lived activation tensors
        # xT chunk-major [e-in-tile, sc, kt, s']: DMA'd straight from the
        # host-pre-transposed x (no on-chip transposes at all)
        xT = big.tile([128, NIC, NKT, 512], BF16, name="xT")
        qT = big.tile([128, NHP, S], BF16, name="qT")   # [d(2 heads), hp, s]
        kT = big.tile([128, NHP, S], BF16, name="kT")
        # v2: [s-in-tile, st, hp, head-in-pair, 2*hd]; cols 64:128 all ones, so
        # the PV matmul replicates the softmax sums across psum rows 64:127
        # (matmul cost depends only on the moving free dim, so this is free)
        # and no partition-broadcast is needed for the normalization.
        v2 = big.tile([128, NST, NHP, 2, 2 * HD], BF16, name="v2")
        nc.gpsimd.memset(v2[:, :, :, :, HD : 2 * HD], 1.0)
        yT = big.tile([128, NPT, S], BF16, name="yT")

        # ---------------- emission helpers ----------------
        def issue_x_dma(sc):
            nc.sync.dma_start(xT[:, sc, :, :], x_h[:][sc])

        cp_rot = [0]

        def emit_qk(which, pt, sc, eng="alt"):
            w_sb, b_sb = (wq_sb, bq_sb) if which == "q" else (wk_sb, bk_sb)
            outT = qT if which == "q" else kT
            acc2 = ps.tile([128, 2, 512], F32, tag="grp", name="acc2")
            acc = acc2[:, 0, :]
            for kt in range(NKT):
                nc.tensor.matmul(
                    acc,
                    lhsT=w_sb[:, kt, pt * 128 : (pt + 1) * 128],
                    rhs=xT[:, sc, kt, :],
                    start=(kt == 0),
                    stop=(kt == NKT - 1),
                )
            dst = outT[:, pt, sc * 512 : (sc + 1) * 512]
            if eng == "alt":
                eng = "act" if cp_rot[0] == 0 else "dve"
                cp_rot[0] = 1 - cp_rot[0]
            if eng == "act":
                nc.scalar.activation(
                    dst, acc, Ident,
                    bias=b_sb[:, pt : pt + 1],
                    scale=float(SCALE) if which == "q" else 1.0,
                )
            elif which == "q":
                # q = acc*scale + bias (bias pre-scaled on host)
                nc.vector.tensor_scalar(
                    dst, acc, float(SCALE), b_sb[:, pt : pt + 1], Mult, Add
                )
            else:
                nc.vector.tensor_scalar_add(dst, acc, b_sb[:, pt : pt + 1])

        def emit_v(st):
            acc2 = ps.tile([128, 2, 512], F32, tag="grp", name="vacc")
            acc = acc2[:, 0, :]
            for kt in range(NKT):
                nc.tensor.matmul(
                    acc,
                    lhsT=xT[:, st // 4, kt, (st % 4) * 128 : (st % 4) * 128 + 128],
                    rhs=wv_sb[:, kt, :],
                    start=(kt == 0),
                    stop=(kt == NKT - 1),
                )
            nc.vector.tensor_add(
                v2[:, st, :, :, 0:HD],
                acc.rearrange("p (a b c) -> p a b c", a=NHP, b=2),
                bvb.rearrange("p (a b c) -> p a b c", a=NHP, b=2),
            )

        def emit_cproj_st(st, direct=False):
            cacc = ps.tile([128, 2, 512], F32, tag="grp", name="cacc")
            for ec in range(2):
                for ptd in range(NPT):
                    nc.tensor.matmul(
                        cacc[:, ec, :],
                        lhsT=yT[:, ptd, st * 128 : (st + 1) * 128],
                        rhs=wo_sb[:, ptd, ec * 512 : (ec + 1) * 512],
                        start=(ptd == 0),
                        stop=(ptd == NPT - 1),
                    )
            if direct:
                # tail path: 4-deep staging from the (now idle) xs pool kills
                # the copy->DMA WAR serialization; alternate Act/DVE copies
                ot = xs_pool.tile([128, E], BF16, tag="xs")
                otv = ot[:].rearrange("p (a b) -> p a b", a=2)
                if st % 2 == 0:
                    nc.scalar.activation(otv, cacc[:], Ident)
                else:
                    nc.vector.tensor_copy(otv, cacc[:])
                nc.scalar.dma_start(out_h[:][st * 128 : (st + 1) * 128, :], ot[:])
            else:
                for ec in range(2):
                    ot = out_pool.tile([128, 512], BF16, tag="ot")
                    nc.vector.tensor_copy(ot[:], cacc[:, ec, :])
                    nc.sync.dma_start(
                        out_h[:][st * 128 : (st + 1) * 128, ec * 512 : (ec + 1) * 512],
                        ot[:],
                    )

        # ---- attention pipeline (global cross-iteration staging) ----
        stage = []          # staged (flush_fn) entries, depth 2
        pending_norm = []   # (yu, hp, ic) awaiting sums partition-shift DMA
        norm_sh = []        # (yu, slo, hp, ic) shift done, recip pending
        norm_rc = []        # (yu, rrb, hp, ic) recip done, muls pending

        def make_flush(jt, ps_s, pt_t, psy, ic, hp, first, last):
            def flush():
                r = jt - 4 * ic
                w0 = 128 * r if r >= 0 else 0
                W = 512 - w0
                nc.scalar.activation(pt_t[:, :, w0:512], ps_s[:, :, 0:W], Exp)
                if r >= 0:
                    for hh in range(2):
                        nc.vector.tensor_mul(
                            pt_t[:, hh, w0 : w0 + 128],
                            pt_t[:, hh, w0 : w0 + 128],
                            trimask[:],
                        )
                for hh in range(2):
                    nc.tensor.matmul(
                        psy[:, hh, w0:512],
                        lhsT=v2[:, jt, hp, hh, :],
                        rhs=pt_t[:, hh, w0:512],
                        start=first,
                        stop=last,
                    )
                if last:
                    # free the PSUM accumulator fast: per-hh copies running in
                    # parallel on Act and DVE; rows 0:64 = y, rows 64:128 =
                    # softmax sums replicated by the v2 ones block
                    yu = yu_pool.tile([128, 2, 512], F32, tag="yu", name="yu")
                    if ic >= 2:
                        nc.vector.tensor_copy(yu[:, 0, :], psy[:, 0, :])
                        nc.vector.tensor_copy(yu[:, 1, :], psy[:, 1, :])
                    else:
                        nc.scalar.activation(yu[:, 0, :], psy[:, 0, :], Ident)
                        nc.vector.tensor_copy(yu[:, 1, :], psy[:, 1, :])
                    pending_norm.append((yu, hp, ic))
            return flush

        def norm_tick():
            # advance the oldest norm-pipeline stage by one step.  DVE ops
            # need both sources on the same base partition (custom recip must
            # be fully unshifted), so a SBUF->SBUF DMA moves the replicated
            # sums rows 64:128 down to base 0 first.
            if norm_rc:
                yu, rrb, hp_, ic_ = norm_rc.pop(0)
                for hh in range(2):
                    nc.vector.tensor_mul(
                        yT[hh * 64 : hh * 64 + 64, hp_, ic_ * 512 : (ic_ + 1) * 512],
                        yu[0:HD, hh, :],
                        rrb[:, hh, :],
                    )
            elif norm_sh:
                yu, slo, hp_, ic_ = norm_sh.pop(0)
                rrb = rb_pool.tile([HD, 2, 512], F32, tag="rrb", name="rrb")
                nc.vector.reciprocal_approx_fast(rrb[:, :, :], slo[:, :, :])
                norm_rc.append((yu, rrb, hp_, ic_))
            elif pending_norm:
                yu, hp_, ic_ = pending_norm.pop(0)
                slo = sl_pool.tile([HD, 2, 512], F32, tag="slo", name="slo")
                nc.vector.tensor_copy(slo[:, :, :], yu[HD : 2 * HD, :, :])
                norm_sh.append((yu, slo, hp_, ic_))

        # ---------------- prologue: sc=0 inputs + weights + proj ----------------
        # DMA issue order is consumption order: the sync queue is FIFO, so x
        # tiles whose staging buffer WARs on a not-yet-run transpose would
        # block later weight loads if issued too early
        _sc = nc.enter_named_scope("fused", False)[0]
        # HAM warm-up: keep the PE continuously busy >3.4us before the first
        # real work so the clock gate opens to 2.4 GHz by the time x arrives.
        # The warm tile is DVE-memset so the warm-ups do NOT wait behind the
        # slow gpsimd queue (trimask select sits after the 7us v2-ones
        # memset there).
        wtile = const.tile([128, 128], BF16, name="wtile")
        nc.vector.memset(wtile[:], 1.0)
        warm = ps.tile([128, 2, 512], F32, tag="grp", name="warm")
        for i in range(32):
            nc.tensor.matmul(
                warm[:, 0, 0:128], lhsT=wtile[:], rhs=wtile[:],
                start=True, stop=True,
            )
        # x chunks on the sync queue (chunk 0 gates the prologue chain);
        # weights stream in parallel on the Act hwdge queue
        # kt-halved first loads: q's kt0-3 matmuls start as soon as the
        # first halves of x chunk 0 and wq land
        nc.sync.dma_start(xT[:, 0, 0:4, :], x_h[:][0, :, 0:4, :])
        nc.scalar.dma_start(wq_sb[:, 0:4, :], wq_h[:][:, 0:4, :])
        nc.sync.dma_start(xT[:, 0, 4:8, :], x_h[:][0, :, 4:8, :])
        nc.scalar.dma_start(wq_sb[:, 4:8, :], wq_h[:][:, 4:8, :])
        nc.scalar.dma_start(wk_sb[:, 0:4, :], wk_h[:][:, 0:4, :])
        issue_x_dma(1)
        nc.sync.dma_start(bz_sb[:], bz_h[:])
        nc.scalar.dma_start(wk_sb[:, 4:8, :], wk_h[:][:, 4:8, :])
        load_weight(wv_h, wv_sb)
        issue_x_dma(2)
        issue_x_dma(3)
        load_weight(wo_h, wo_sb)
        for pt in range(NPT):
            emit_qk("q", pt, 0)
        for pt in range(NPT):
            emit_qk("k", pt, 0)
        for st in range(4):
            emit_v(st)

        # ---------------- fused attention + next-chunk projection ----------------
        for ic in range(NIC):
            items = []
            if ic < 3:
                scn = ic + 1
                eng = "alt" if ic < 2 else "dve"
                for pt in range(NPT):
                    items.append(
                        lambda pt=pt, scn=scn, eng=eng: emit_qk("q", pt, scn, eng)
                    )
                for pt in range(NPT):
                    items.append(
                        lambda pt=pt, scn=scn, eng=eng: emit_qk("k", pt, scn, eng)
                    )
                for st in range(4 * scn, 4 * scn + 4):
                    items.append(lambda st=st: emit_v(st))
            if ic == 2:
                for st in range(0, 4):
                    items.append(lambda st=st: emit_cproj_st(st))
            if ic == 3:
                for st in range(4, 12):
                    items.append(lambda st=st: emit_cproj_st(st))

            # Scores are emitted in PAIRS of steps, then the two staged
            # flushes for the previous pair run.  Pairing gives every
            # LDWEIGHTS a drained weight-buffer slot to prefetch into
            # (consecutive full-width/row-tiled stationaries otherwise
            # serialize LDW behind the in-flight matmul drain).
            # Item emission between pairs only (not at pair boundaries near
            # the psy drain).
            njt = 4 * ic + 4
            npair = njt // 2
            n_allowed = NHP * npair
            adone = 0
            emitted = 0
            for hp in range(NHP):
                psy = ps_y.tile([128, 2, 512], F32, tag="y", name="psy")
                for pj in range(npair):
                    for jt in (2 * pj, 2 * pj + 1):
                        pos = jt
                        r = jt - 4 * ic
                        w0 = 128 * r if r >= 0 else 0
                        W = 512 - w0
                        ps_s = ps.tile([128, 2, 512], F32, tag="grp")
                        for hh in range(2):
                            base = hh * 64
                            nc.tensor.matmul(
                                ps_s[:, hh, 0:W],
                                lhsT=kT[base : base + 64, hp, jt * 128 : (jt + 1) * 128],
                                rhs=qT[base : base + 64, hp, ic * 512 + w0 : (ic + 1) * 512],
                                start=True,
                                stop=True,
                            )
                        pt_t = pt_pool.tile([128, 2, 512], BF16, tag="pt")
                        stage.append(
                            make_flush(jt, ps_s, pt_t, psy, ic, hp,
                                       pos == 0, pos == njt - 1)
                        )
                    while len(stage) > 2:
                        stage.pop(0)()
                    norm_tick()
                    if True:
                        adone += 1
                        target = (adone * len(items) + n_allowed - 1) // n_allowed
                        while items and emitted < target:
                            items[emitted]()
                            emitted += 1
            while items and emitted < len(items):
                items[emitted]()
                emitted += 1

        # drain: overlap the last head-pair's norm chain with the tail
        # cproj by pre-accumulating the ptd<3 contributions (hp0-2 normed).
        # st15 borrows the freed psy bank; final DMAs split across queues.
        while stage:
            stage.pop(0)()
        norm_tick()
        norm_tick()
        cacc_t = {}
        for st in (12, 13, 14, 15):
            pool_ = ps if st < 15 else ps_y
            cacc = pool_.tile([128, 2, 512], F32, tag="grp" if st < 15 else "y",
                              name="cacc")
            cacc_t[st] = cacc
            for ec in range(2):
                for ptd in range(3):
                    nc.tensor.matmul(
                        cacc[:, ec, :],
                        lhsT=yT[:, ptd, st * 128 : (st + 1) * 128],
                        rhs=wo_sb[:, ptd, ec * 512 : (ec + 1) * 512],
                        start=(ptd == 0),
                        stop=False,
                    )
            norm_tick()
        while pending_norm or norm_sh or norm_rc:
            norm_tick()
        for st in (12, 13, 14, 15):
            cacc = cacc_t[st]
            for ec in range(2):
                nc.tensor.matmul(
                    cacc[:, ec, :],
                    lhsT=yT[:, 3, st * 128 : (st + 1) * 128],
                    rhs=wo_sb[:, 3, ec * 512 : (ec + 1) * 512],
                    start=False,
                    stop=True,
                )
            ot = xs_pool.tile([128, E], BF16, tag="xs")
            otv = ot[:].rearrange("p (a b) -> p a b", a=2)
            if st % 2 == 0:
                nc.scalar.activation(otv, cacc[:], Ident)
            else:
                nc.vector.tensor_copy(otv, cacc[:])
            deng = nc.sync if st % 2 == 0 else nc.scalar
            deng.dma_start(out_h[:][st * 128 : (st + 1) * 128, :], ot[:])
        nc.leave_named_scope("fused", _sc, False)

def _get_nc(mode="mixed"):
    if mode not in _CACHED_NC:
        _CACHED_NC[mode] = build_bass(mode)
    return _CACHED_NC[mode]


def make_in_maps(x, Wq, bq, Wk, bk, Wv, bv, Wo, bo):
    import ml_dtypes

    bf16 = ml_dtypes.bfloat16
    in_maps = []
    for c in range(NCORES):
        b = c % B
        half = c // B
        sl = slice(half * EH, (half + 1) * EH)
        in_maps.append(
            {
                "x": np.ascontiguousarray(
                    x[b]
                    .reshape(NIC, 512, NKT, 128)
                    .transpose(0, 3, 2, 1)
                ).astype(bf16),
                "wq": np.ascontiguousarray(
                    Wq[:, sl].reshape(NKT, 128, EH).transpose(1, 0, 2)
                ).astype(bf16),
                "wk": np.ascontiguousarray(
                    Wk[:, sl].reshape(NKT, 128, EH).transpose(1, 0, 2)
                ).astype(bf16),
                "wv": np.ascontiguousarray(
                    Wv[:, sl].reshape(NKT, 128, EH).transpose(1, 0, 2)
                ).astype(bf16),
                "wo": np.ascontiguousarray(
                    Wo[sl, :].reshape(NPT, 128, E).transpose(1, 0, 2)
                ).astype(bf16),
                "bz": np.concatenate(
                    [
                        (bq[sl] * np.float32(SCALE)).reshape(NPT, 128).T,
                        bk[sl].reshape(NPT, 128).T,
                        np.tile(bv[sl], (128, 1)),
                    ],
                    axis=1,
                ).astype(np.float32),
            }
        )
    return in_maps


def assemble(results, bo):
    out = np.empty((B, S, E), dtype=np.float32)
    for b in range(B):
        out[b] = (
            results[b]["out"].astype(np.float32)
            + results[b + B]["out"].astype(np.float32)
            + bo[None, :]
        )
    return out


def kernel(x, Wq, bq, Wk, bk, Wv, bv, Wo, bo, _trace=False, _mode="mixed"):
    x = np.asarray(x, dtype=np.float32)
    Wq = np.asarray(Wq, dtype=np.float32)
    bq = np.asarray(bq, dtype=np.float32)
    Wk = np.asarray(Wk, dtype=np.float32)
    bk = np.asarray(bk, dtype=np.float32)
    Wv = np.asarray(Wv, dtype=np.float32)
    bv = np.asarray(bv, dtype=np.float32)
    Wo = np.asarray(Wo, dtype=np.float32)
    bo = np.asarray(bo, dtype=np.float32)

    nc = _get_nc(_mode)
    in_maps = make_in_maps(x, Wq, bq, Wk, bk, Wv, bv, Wo, bo)
    res = run_bass_kernel_spmd(nc, in_maps, list(range(NCORES)), trace=_trace)
    out = assemble(res.results, bo)
    if _trace:
        return out, res
    return out

